# revision 1
# baseline (speedup 1.0000x reference)
"""Attentional pooling layer on Trainium2 (Bass/Tile), 8-core batch-parallel.

Reference computation per batch b:
    scores[hw, n] = sum_c f[c, hw] * w[c, n]          (mm1, fp32)
    num           = softplus(scores)                  (ACT: Abs/Exp/Ln)
    denom[n]      = sum_hw num[hw, n] + 16*CONST      (PE reduce + DVE)
    att[hw, n]    = (num + CONST) / denom[n]          (PE bcast + DVE)
    out[c, n]     = sum_hw f[c, hw] * att[hw, n]      (mm2, float32r)

Partition layout: 3 batches are packed into one 96-partition group at
32-partition offsets (PE tile_position only supports 32-aligned output
partition bases 0/32/64 for small-M matmuls).  mm1 runs M=32 with
zero-padded feature columns so the 16 garbage rows per 32-block are written
with clean zeros.  Partition-dim reductions (sum over hw) and broadcasts
(denom over hw) are done with tiny constant 0/1 matmuls (bd / exp3) fed
from host numpy.  The col-0 matmuls (denominator reduce, broadcast, mm2)
run as float32r (TF32, full PE rate); their operands are rounded to f32r by
the producing ACT/DVE ops.  mm1 stays fp32 (f32r cannot write PSUM at a
nonzero partition base).

32 batches per core = 10 groups of 3 + one ragged group [30, 31, 30] where
the duplicated slot's output is skipped.
"""

import numpy as np
from contextlib import ExitStack

import concourse.bass as bass
import concourse.bacc as bacc
import concourse.tile as tile
from concourse import mybir
from concourse.bass_utils import run_bass_kernel_spmd

F32 = mybir.dt.float32
F32R = mybir.dt.float32r
AF = mybir.ActivationFunctionType
ALU = mybir.AluOpType

N_CORES = 8
B_FULL, C, H, W, N = 256, 256, 4, 4, 2048
HW = H * W                  # 16
B = B_FULL // N_CORES       # 32 batches per core
KC = C // 128               # 2 contraction chunks of 128
GB = 3                      # batches per partition group (32-part offsets 0/32/64)
GP = 32 * GB                # 96 partitions used per group
NCH = 4                     # n chunks per group chain
NW = N // NCH               # 512 (one PSUM bank)
CONST = 1e-4


def make_groups(n_batch):
    """Chunks of GB batches; ragged tail padded with duplicates (emit=False)."""
    groups = []
    for s in range(0, n_batch, GB):
        real = list(range(s, min(s + GB, n_batch)))
        emit = [True] * len(real)
        while len(real) < GB:
            real.append(real[0])
            emit.append(False)
        groups.append((real, emit))
    return groups


def aux_inputs():
    # bd[k, m] = 1 iff row k is one of batch-slot m's real hw rows
    bd = np.zeros((GP, GB), np.float32)
    for k in range(GP):
        if k % 32 < HW:
            bd[k, k // 32] = 1.0
    # exp3[k, m] = 1 iff partition m belongs to batch-slot k's 32-block
    exp3 = np.zeros((GB, GP), np.float32)
    for m in range(GP):
        exp3[m // 32, m] = 1.0
    iden = np.eye(128, dtype=np.float32)
    return {"bd": bd, "exp3": exp3, "iden": iden}


def build_nc(n_batch=B, debug=False):
    nc = bacc.Bacc(None, target_bir_lowering=False, debug=debug)
    feat = nc.dram_tensor("fpad", [128, KC, n_batch, 32], F32, kind="ExternalInput")
    wts = nc.dram_tensor("weights", [n_batch, C, N], F32, kind="ExternalInput")
    out = nc.dram_tensor("out", [n_batch, C, N], F32, kind="ExternalOutput")
    bd_d = nc.dram_tensor("bd", [GP, GB], F32R, kind="ExternalInput")
    exp_d = nc.dram_tensor("exp3", [GB, GP], F32R, kind="ExternalInput")
    id_d = nc.dram_tensor("iden", [128, 128], F32, kind="ExternalInput")

    # [ci, b, kc, n] views of the DRAM tensors
    wts_r = wts.ap().rearrange("b (kc ci) n -> ci b kc n", kc=KC)
    out_r = out.ap().rearrange("b (kc ci) n -> ci b kc n", kc=KC)

    with tile.TileContext(nc) as tc, ExitStack() as ctx:
        singles = ctx.enter_context(tc.tile_pool(name="singles", bufs=1))
        wpool = ctx.enter_context(tc.tile_pool(name="w", bufs=5))
        opool = ctx.enter_context(tc.tile_pool(name="o", bufs=3))
        numpool = ctx.enter_context(tc.tile_pool(name="num", bufs=3))
        attpool = ctx.enter_context(tc.tile_pool(name="att", bufs=2))
        smallpool = ctx.enter_context(tc.tile_pool(name="small", bufs=3))
        ftpool = ctx.enter_context(tc.tile_pool(name="ft", bufs=2))
        ps_sc = ctx.enter_context(tc.tile_pool(name="ps_sc", bufs=4, space="PSUM"))
        ps_dr = ctx.enter_context(tc.tile_pool(name="ps_dr", bufs=1, space="PSUM"))
        ps_ft = ctx.enter_context(tc.tile_pool(name="ps_ft", bufs=1, space="PSUM"))
        ps_o = ctx.enter_context(tc.tile_pool(name="ps_o", bufs=2, space="PSUM"))

        bd_t = singles.tile([GP, GB], F32R)
        nc.sync.dma_start(out=bd_t, in_=bd_d.ap())
        exp_t = singles.tile([GB, GP], F32R)
        nc.sync.dma_start(out=exp_t, in_=exp_d.ap())
        id_t = singles.tile([128, 128], F32)
        nc.sync.dma_start(out=id_t, in_=id_d.ap())

        # features, pre-transposed + hw-padded to 32 with zeros on the host
        f_t = singles.tile([128, KC, n_batch, 32], F32)
        nc.sync.dma_start(out=f_t, in_=feat.ap())

        ev = 0
        for bs, emit in make_groups(n_batch):
            w_t = {}
            for b in set(bs):
                w_t[b] = wpool.tile([128, KC, N], F32, tag="w", name="w_t")
                nc.sync.dma_start(out=w_t[b], in_=wts_r[:, b])

            # transposed features fT[hw, c] for mm2.  Transposing the full
            # zero-padded [128, nreal, 32] slice puts slot j's fT at
            # partition 32j (transpose outputs must start at partition 0).
            nreal = len(set(bs))
            ft_ps = ps_ft.tile([32 * nreal, KC, 128], F32, name="ft_ps")
            for kc in range(KC):
                nc.tensor.transpose(
                    ft_ps[:, kc, :],
                    f_t[:, kc, bs[0] : bs[0] + nreal, :],
                    id_t,
                )
            ft_sb = ftpool.tile([32 * nreal, KC, 128], F32R, name="ft_sb")
            nc.scalar.copy(ft_sb, ft_ps)

            att_t = attpool.tile([GP, NCH, NW], F32R)
            # mm1 for all chunks first, then phase the ACT work (all Abs+Exp,
            # then all Lns) so the table-set switches happen twice per group
            # instead of twice per chunk; explicit deps pin the ACT order.
            sc_l, te_l, tl_l = [], [], []
            for nb in range(NCH):
                sc_ps = ps_sc.tile([GP, NW], F32, name="sc_ps")
                for j in range(GB):
                    for kc in range(KC):
                        nc.tensor.matmul(
                            sc_ps[32 * j : 32 * j + 32, :],
                            f_t[:, kc, bs[j], :],
                            w_t[bs[j]][:, kc, nb * NW : (nb + 1) * NW],
                            start=(kc == 0),
                            stop=(kc == KC - 1),
                        )
                sc_l.append(sc_ps)
            # softplus(x) = max(x,0) + ln(1 + exp(-|x|)): exp arg <= 0 so no
            # overflow, Ln input stays in [1,2]
            exp_insts = []
            for nb in range(NCH):
                t_abs = numpool.tile([GP, NW], F32, tag="tabs")
                nc.scalar.activation(t_abs, sc_l[nb], AF.Abs)
                t_exp = numpool.tile([GP, NW], F32, tag="texp", bufs=NCH)
                exp_insts.append(
                    nc.scalar.activation(t_exp, t_abs, AF.Exp, scale=-1.0)
                )
                te_l.append(t_exp)
            for nb in range(NCH):
                t_ln = numpool.tile([GP, NW], F32, tag="tln", bufs=NCH)
                ln_i = nc.scalar.activation(t_ln, te_l[nb], AF.Ln, bias=1.0)
                tile.add_dep_helper(
                    ln_i.ins, exp_insts[-1].ins, sync=False,
                    reason="cluster Lns after all Exps (one table switch)",
                )
                tl_l.append(t_ln)
            for nb in range(NCH):
                num_t = numpool.tile([GP, NW], F32R, tag="num")
                nc.vector.scalar_tensor_tensor(
                    num_t, sc_l[nb], 0.0, tl_l[nb], op0=ALU.max, op1=ALU.add
                )
                d_ps = ps_dr.tile([GB, NW], F32, tag="dr", name="d_ps")
                nc.tensor.matmul(
                    d_ps,
                    bd_t,
                    num_t,
                    start=True,
                    stop=True,
                )
                r_t = smallpool.tile([GB, NW], F32R)
                with nc.allow_low_precision(reason="tf32 matmul operand"):
                    nc.vector.tensor_scalar_add(r_t, d_ps, HW * CONST)
                    nc.vector.reciprocal(r_t, r_t)
                rb_ps = ps_dr.tile([GP, NW], F32, tag="dr", name="rb_ps")
                nc.tensor.matmul(
                    rb_ps,
                    exp_t,
                    r_t,
                    start=True,
                    stop=True,
                )
                # att = (num + CONST) * (1/denom)
                nc.vector.scalar_tensor_tensor(
                    att_t[:, nb, :],
                    num_t,
                    CONST,
                    rb_ps,
                    op0=ALU.add,
                    op1=ALU.mult,
                )

            for j in range(GB):
                if not emit[j]:
                    continue
                o_sb = opool.tile([128, KC, N], F32, tag="o", name="o_sb")
                for kc in range(KC):
                    for nb in range(NCH):
                        o_ps = ps_o.tile([128, NW], F32)
                        nc.tensor.matmul(
                            o_ps,
                            ft_sb[32 * j : 32 * j + HW, kc, :],
                            att_t[32 * j : 32 * j + HW, nb, :],
                            start=True,
                            stop=True,
                        )
                        dst = o_sb[:, kc, nb * NW : (nb + 1) * NW]
                        if ev % 2 == 0:
                            nc.vector.tensor_copy(dst, o_ps)
                        else:
                            nc.scalar.copy(dst, o_ps)
                        ev += 1
                nc.sync.dma_start(out=out_r[:, bs[j]], in_=o_sb)

    nc.compile()
    return nc


_NC_CACHE = {}


def _get_nc(n_batch=B):
    if n_batch not in _NC_CACHE:
        _NC_CACHE[n_batch] = build_nc(n_batch)
    return _NC_CACHE[n_batch]


def prep_features(features, dtype=np.float32):
    """[nb, C, H, W] f32 -> padded [128, KC, nb, 32] in dtype."""
    features = np.asarray(features).astype(dtype)
    nb = features.shape[0]
    f4 = features.reshape(nb, KC, 128, HW)
    fpad = np.zeros((nb, KC, 128, 32), dtype)
    fpad[..., :HW] = f4
    return np.ascontiguousarray(fpad.transpose(2, 1, 0, 3))  # [128, KC, nb, 32]


def run(features, weights, trace=False, **kwargs):
    """Shard over 8 cores, run, gather. Returns (out, BassKernelResults)."""
    fpad = prep_features(features)
    weights = np.ascontiguousarray(np.asarray(weights), dtype=np.float32)
    aux = aux_inputs()
    nc = _get_nc()
    in_maps = []
    for i in range(N_CORES):
        sl = slice(i * B, (i + 1) * B)
        in_maps.append(
            {"fpad": fpad[:, :, sl], "weights": weights[sl], **aux}
        )
    res = run_bass_kernel_spmd(
        nc, in_maps, core_ids=list(range(N_CORES)), trace=trace, **kwargs
    )
    out = np.concatenate([r["out"] for r in res.results], axis=0).astype(np.float32)
    return out, res


def kernel(features, weights):
    out, _ = run(features, weights)
    return out



# revision 2
# speedup vs baseline: 2.1121x; 2.1121x over previous
"""Attentional pooling layer on Trainium2 (Bass/Tile), 8-core batch-parallel.

Reference computation per batch b:
    scores[hw, n] = sum_c f[c, hw] * w[c, n]          (mm1)
    num           = softplus(scores)                  (ACT: Abs/Exp/Ln)
    denom[n]      = sum_hw num[hw, n]                 (PE reduce)
    att[hw, n]    = num / denom[n]                    (DVE recip + mult)
    out[c, n]     = sum_hw f[c, hw] * att[hw, n]      (mm2)

All dense traffic is bf16 (weights in, outputs out) — the rel-err budget
(2e-2) dwarfs bf16 rounding (~4e-3 end to end), and HBM time is the
bottleneck, so halving bytes halves the roofline.  The +1e-4 numerator /
+16e-4 denominator constants of the reference are dropped entirely
(measured end-to-end impact 5.5e-5).

Partition layout: 4 batches per group at 32-partition offsets 0/32/64/96
(explicit tile_position, bypassing the conservative 0/32/64 helper assert;
base 96 verified on hardware).  32 batches per core = 8 exact groups.

The denominator reduce-over-hw AND its broadcast back to all 128
partitions are fused into ONE matmul with a constant 0/1 matrix
C[k, m] = 1 iff k, m in the same 32-block and k%32 < 16.  A single act
table (natural_log_exp_and_others: Abs+Exp+Ln+Copy) serves every ACT op,
so exactly one table load is emitted — forced by blanking the other table
entries (indices preserved, so act_func_set_id still matches walrus's
act_info.json).

Output drain PSUM->SBUF casts f32->bf16 on DVE/ACT (GPSIMD cannot touch
PSUM), in [128,1024] units (2 PSUM banks) to amortize access latency.
"""

import numpy as np
import ml_dtypes
from contextlib import ExitStack

import concourse.bass as bass
import concourse.bacc as bacc
import concourse.tile as tile
from concourse import mybir
from concourse.bass_utils import run_bass_kernel_spmd

F32 = mybir.dt.float32
BF16 = mybir.dt.bfloat16
AF = mybir.ActivationFunctionType
ALU = mybir.AluOpType
BF_NP = ml_dtypes.bfloat16

# Force every ACT op onto one table so only a single table load is emitted.
# Keys/positions are preserved (ids index walrus's act_info.json); only the
# *selection* changes: all other tables are presented as empty so the greedy
# insert pass picks natural_log_exp_and_others (Abs+Exp+Ln+Copy) for all.
_KEEP_TABLE = "natural_log_exp_and_others"
_orig_get_tables = bacc.get_activation_tables


def _single_table(arch):
    tbls = _orig_get_tables(arch)
    return {k: (v if k == _KEEP_TABLE else set()) for k, v in tbls.items()}


bacc.get_activation_tables = _single_table

N_CORES = 8
B_FULL, C, H, W, N = 256, 256, 4, 4, 2048
HW = H * W                  # 16
B = B_FULL // N_CORES       # 32 batches per core
KC = C // 128               # 2 contraction chunks of 128
GB = 4                      # batches per partition group (offsets 0/32/64/96)
GP = 32 * GB                # 128 partitions per group
NCH = 4                     # n chunks per group
NW = N // NCH               # 512 (one PSUM bank)


def aux_inputs():
    # cmat[k, m] = 1 iff k and m share a 32-block and k%32 is a real hw row;
    # (C^T @ num) both reduces over hw and broadcasts back to 128 partitions.
    cmat = np.zeros((GP, GP), np.float32)
    for k in range(GP):
        for m in range(GP):
            if k // 32 == m // 32 and k % 32 < HW:
                cmat[k, m] = 1.0
    iden = np.eye(128, dtype=np.float32)
    return {"cmat": cmat.astype(BF_NP), "iden": iden.astype(BF_NP)}


def build_nc(n_batch=B, debug=False):
    nc = bacc.Bacc(None, target_bir_lowering=False, debug=debug)
    feat = nc.dram_tensor("fpad", [128, KC, n_batch, 32], BF16, kind="ExternalInput")
    wts = nc.dram_tensor("weights", [n_batch, C, N], BF16, kind="ExternalInput")
    out = nc.dram_tensor("out", [n_batch, C, N], BF16, kind="ExternalOutput")
    cm_d = nc.dram_tensor("cmat", [GP, GP], BF16, kind="ExternalInput")
    id_d = nc.dram_tensor("iden", [128, 128], BF16, kind="ExternalInput")

    # [ci, b, kc, n] views of the DRAM tensors
    wts_r = wts.ap().rearrange("b (kc ci) n -> ci b kc n", kc=KC)
    out_r = out.ap().rearrange("b (kc ci) n -> ci b kc n", kc=KC)

    n_groups = (n_batch + GB - 1) // GB

    with tile.TileContext(nc) as tc, ExitStack() as ctx:
        singles = ctx.enter_context(tc.tile_pool(name="singles", bufs=1))
        wpool = ctx.enter_context(tc.tile_pool(name="w", bufs=8))
        opool = ctx.enter_context(tc.tile_pool(name="o", bufs=3))
        sppool = ctx.enter_context(tc.tile_pool(name="sp", bufs=2))
        numpool = ctx.enter_context(tc.tile_pool(name="num", bufs=2))
        rpool = ctx.enter_context(tc.tile_pool(name="r", bufs=2))
        attpool = ctx.enter_context(tc.tile_pool(name="att", bufs=2))
        ftpool = ctx.enter_context(tc.tile_pool(name="ft", bufs=2))
        ps_sc = ctx.enter_context(tc.tile_pool(name="ps_sc", bufs=2, space="PSUM"))
        ps_d = ctx.enter_context(tc.tile_pool(name="ps_d", bufs=1, space="PSUM"))
        ps_ft = ctx.enter_context(tc.tile_pool(name="ps_ft", bufs=1, space="PSUM"))
        ps_o = ctx.enter_context(tc.tile_pool(name="ps_o", bufs=2, space="PSUM"))

        cm_t = singles.tile([GP, GP], BF16)
        nc.sync.dma_start(out=cm_t, in_=cm_d.ap())
        id_t = singles.tile([128, 128], BF16)
        nc.sync.dma_start(out=id_t, in_=id_d.ap())

        # features, pre-transposed + hw-padded to 32 with zeros on the host
        f_t = singles.tile([128, KC, n_batch, 32], BF16)
        nc.sync.dma_start(out=f_t, in_=feat.ap())

        ev = 0
        for g in range(n_groups):
            bs = [min(g * GB + j, n_batch - 1) for j in range(GB)]
            emit = [g * GB + j < n_batch for j in range(GB)]
            w_t = {}
            for b in sorted(set(bs)):
                w_t[b] = wpool.tile([128, KC, N], BF16, tag="w", name="w_t")
                nc.sync.dma_start(out=w_t[b], in_=wts_r[:, b])

            # transposed features fT[hw, c] for mm2, one 128x128 transpose
            # per kc (slot j's fT lands at partition 32j automatically)
            ft_ps = ps_ft.tile([GP, KC, 128], BF16, name="ft_ps")
            for kc in range(KC):
                nc.tensor.transpose(
                    ft_ps[:, kc, :],
                    f_t[:, kc, bs[0] : bs[0] + GB, :],
                    id_t,
                )
            ft_sb = ftpool.tile([GP, KC, 128], BF16, name="ft_sb")
            nc.scalar.copy(ft_sb, ft_ps)

            att_t = attpool.tile([GP, NCH, NW], BF16)
            for nb in range(NCH):
                sc_ps = ps_sc.tile([GP, NW], F32, name="sc_ps")
                for j in range(GB):
                    for kc in range(KC):
                        nc.tensor.matmul(
                            sc_ps[32 * j : 32 * j + 32, :],
                            f_t[:, kc, bs[j], :],
                            w_t[bs[j]][:, kc, nb * NW : (nb + 1) * NW],
                            start=(kc == 0),
                            stop=(kc == KC - 1),
                            tile_position=(0, 32 * j),
                        )
                # softplus(x) = max(x,0) + ln(1 + exp(-|x|)): exp arg <= 0 so
                # no overflow, Ln input stays in [1,2]
                t_abs = sppool.tile([GP, NW], F32, tag="tabs")
                nc.scalar.activation(t_abs, sc_ps, AF.Abs)
                t_exp = sppool.tile([GP, NW], F32, tag="texp")
                nc.scalar.activation(t_exp, t_abs, AF.Exp, scale=-1.0)
                t_ln = sppool.tile([GP, NW], F32, tag="tln")
                nc.scalar.activation(t_ln, t_exp, AF.Ln, bias=1.0)
                num_t = numpool.tile([GP, NW], BF16, tag="num")
                nc.vector.scalar_tensor_tensor(
                    num_t, sc_ps, 0.0, t_ln, op0=ALU.max, op1=ALU.add
                )
                # fused reduce-over-hw + broadcast: d[m] = sum_k C[k,m] num[k]
                d_ps = ps_d.tile([GP, NW], F32, name="d_ps")
                nc.tensor.matmul(d_ps, cm_t, num_t, start=True, stop=True)
                r_t = rpool.tile([GP, NW], F32, tag="r")
                nc.vector.reciprocal(r_t, d_ps)
                nc.vector.tensor_tensor(
                    att_t[:, nb, :], num_t, r_t, op=ALU.mult
                )

            for j in range(GB):
                if not emit[j]:
                    continue
                o_sb = opool.tile([128, KC, N], BF16, tag="o", name="o_sb")
                for kc in range(KC):
                    for h in range(2):
                        o_ps = ps_o.tile([128, 2, NW], F32)
                        for q in range(2):
                            nb = 2 * h + q
                            nc.tensor.matmul(
                                o_ps[:, q, :],
                                ft_sb[32 * j : 32 * j + HW, kc, :],
                                att_t[32 * j : 32 * j + HW, nb, :],
                                start=True,
                                stop=True,
                                tile_position=(32 * j, 0),
                            )
                        dst = o_sb[:, kc, 2 * h * NW : 2 * h * NW + 2 * NW]
                        if ev % 2 == 0:
                            nc.scalar.copy(dst, o_ps)
                        else:
                            nc.vector.tensor_copy(dst, o_ps)
                        ev += 1
                nc.sync.dma_start(out=out_r[:, bs[j]], in_=o_sb)

    nc.compile()
    return nc


_NC_CACHE = {}


def _get_nc(n_batch=B):
    if n_batch not in _NC_CACHE:
        _NC_CACHE[n_batch] = build_nc(n_batch)
    return _NC_CACHE[n_batch]


def prep_features(features):
    """[nb, C, H, W] f32 -> padded bf16 [128, KC, nb, 32]."""
    features = np.asarray(features, dtype=np.float32)
    nb = features.shape[0]
    f4 = features.reshape(nb, KC, 128, HW)
    fpad = np.zeros((nb, KC, 128, 32), np.float32)
    fpad[..., :HW] = f4
    return np.ascontiguousarray(fpad.transpose(2, 1, 0, 3)).astype(BF_NP)


def run(features, weights, trace=False, **kwargs):
    """Shard over 8 cores, run, gather. Returns (out, BassKernelResults)."""
    fpad = prep_features(features)
    weights = np.asarray(weights, dtype=np.float32).astype(BF_NP)
    aux = aux_inputs()
    nc = _get_nc()
    in_maps = []
    for i in range(N_CORES):
        sl = slice(i * B, (i + 1) * B)
        in_maps.append({"fpad": fpad[:, :, sl], "weights": weights[sl], **aux})
    res = run_bass_kernel_spmd(
        nc, in_maps, core_ids=list(range(N_CORES)), trace=trace, **kwargs
    )
    out = np.concatenate(
        [np.asarray(r["out"]).astype(np.float32) for r in res.results], axis=0
    )
    return out, res


def kernel(features, weights):
    out, _ = run(features, weights)
    return out


# revision 46
# speedup vs baseline: 2.4402x; 1.1554x over previous
"""Attentional pooling layer on Trainium2 (Bass/Tile), 8-core batch-parallel.

Reference computation per batch b:
    scores[hw, n] = sum_c f[c, hw] * w[c, n]          (mm1)
    num           = softplus(scores)                  (ACT: Abs/Exp/Ln)
    denom[n]      = sum_hw num[hw, n]                 (PE reduce)
    att[hw, n]    = num / denom[n]                    (DVE recip + mult)
    out[c, n]     = sum_hw f[c, hw] * att[hw, n]      (mm2)

All dense traffic is bf16 (weights in, outputs out) — the rel-err budget
(2e-2) dwarfs bf16 rounding (~4e-3 end to end), and HBM time is the
bottleneck, so halving bytes halves the roofline.  The +1e-4 numerator /
+16e-4 denominator constants of the reference are dropped entirely
(measured end-to-end impact 5.5e-5).

Partition layout: 4 batches per group at 32-partition offsets 0/32/64/96
(explicit tile_position, bypassing the conservative 0/32/64 helper assert;
base 96 verified on hardware).  32 batches per core = 8 exact groups.

The denominator reduce-over-hw AND its broadcast back to all 128
partitions are fused into ONE matmul with a constant 0/1 matrix
C[k, m] = 1 iff k, m in the same 32-block and k%32 < 16.  A single act
table (natural_log_exp_and_others: Abs+Exp+Ln+Copy) serves every ACT op,
so exactly one table load is emitted — forced by blanking the other table
entries (indices preserved, so act_func_set_id still matches walrus's
act_info.json).

Output drain PSUM->SBUF casts f32->bf16 on DVE/ACT (GPSIMD cannot touch
PSUM), in [128,1024] units (2 PSUM banks) to amortize access latency.
"""

import numpy as np
import ml_dtypes
from contextlib import ExitStack

import concourse.bass as bass
import concourse.bacc as bacc
import concourse.tile as tile
from concourse import mybir
from concourse.bass_utils import run_bass_kernel_spmd

F32 = mybir.dt.float32
BF16 = mybir.dt.bfloat16
AF = mybir.ActivationFunctionType
ALU = mybir.AluOpType
BF_NP = ml_dtypes.bfloat16

# Force every ACT op onto one table so only a single table load is emitted.
# Keys/positions are preserved (ids index walrus's act_info.json); only the
# *selection* changes: all other tables are presented as empty so the greedy
# insert pass picks natural_log_exp_and_others (Abs+Exp+Ln+Copy) for all.
_KEEP_TABLE = "natural_log_exp_and_others"
_orig_get_tables = bacc.get_activation_tables


def _single_table(arch):
    tbls = _orig_get_tables(arch)
    return {k: (v if k == _KEEP_TABLE else set()) for k, v in tbls.items()}


bacc.get_activation_tables = _single_table

N_CORES = 8
B_FULL, C, H, W, N = 256, 256, 4, 4, 2048
HW = H * W                  # 16
B = B_FULL // N_CORES       # 32 batches per core
KC = C // 128               # 2 contraction chunks of 128
GB = 4                      # batches per partition group (offsets 0/32/64/96)
GP = 32 * GB                # 128 partitions per group
NCH = 4                     # n chunks per group
NW = N // NCH               # 512 (one PSUM bank)


def aux_inputs():
    # aux[:, 0, :] = cmat: cmat[k, m] = 1 iff k and m share a 32-block and
    # k%32 is a real hw row; (C^T @ num) both reduces over hw and broadcasts
    # back to 128 partitions.  aux[:, 1, :] = 128x128 identity (transposes).
    cmat = np.zeros((GP, GP), np.float32)
    for k in range(GP):
        for m in range(GP):
            if k // 32 == m // 32 and k % 32 < HW:
                cmat[k, m] = 1.0
    aux = np.stack([cmat, np.eye(128, dtype=np.float32)], axis=1)
    return {"aux": aux.astype(BF_NP)}


def build_nc(n_batch=B, debug=False):
    nc = bacc.Bacc(None, target_bir_lowering=False, debug=debug)
    feat = nc.dram_tensor("fpad", [128, KC, n_batch, 32], BF16, kind="ExternalInput")
    wts = nc.dram_tensor("weights", [n_batch, C, N], BF16, kind="ExternalInput")
    out = nc.dram_tensor("out", [n_batch, C, N], BF16, kind="ExternalOutput")
    aux_d = nc.dram_tensor("aux", [128, 2, 128], BF16, kind="ExternalInput")

    # [ci, b, kc, n] views of the DRAM tensors
    wts_r = wts.ap().rearrange("b (kc ci) n -> ci b kc n", kc=KC)
    out_r = out.ap().rearrange("b (kc ci) n -> ci b kc n", kc=KC)

    n_groups = (n_batch + GB - 1) // GB

    with tile.TileContext(nc) as tc, ExitStack() as ctx:
        singles = ctx.enter_context(tc.tile_pool(name="singles", bufs=1))
        wpool = ctx.enter_context(tc.tile_pool(name="w", bufs=12))
        opool = ctx.enter_context(tc.tile_pool(name="o", bufs=6))
        stashpool = ctx.enter_context(tc.tile_pool(name="stash", bufs=29))
        stash = []
        gate_inst = None
        gate2_inst = None
        first_mm1 = {}
        sppool = ctx.enter_context(tc.tile_pool(name="sp", bufs=2))
        numpool = ctx.enter_context(tc.tile_pool(name="num", bufs=2))
        rpool = ctx.enter_context(tc.tile_pool(name="r", bufs=2))
        attpool = ctx.enter_context(tc.tile_pool(name="att", bufs=2))
        ftpool = ctx.enter_context(tc.tile_pool(name="ft", bufs=2))
        ps_sc = ctx.enter_context(tc.tile_pool(name="ps_sc", bufs=2, space="PSUM"))
        ps_d = ctx.enter_context(tc.tile_pool(name="ps_d", bufs=1, space="PSUM"))
        ps_ft = ctx.enter_context(tc.tile_pool(name="ps_ft", bufs=1, space="PSUM"))
        ps_o = ctx.enter_context(tc.tile_pool(name="ps_o", bufs=2, space="PSUM"))

        # features, pre-transposed + hw-padded to 32 with zeros on the host
        f_t = singles.tile([128, KC, n_batch, 32], BF16)
        nc.sync.dma_start(out=f_t, in_=feat.ap())

        aux_t = singles.tile([128, 2, 128], BF16)
        nc.sync.dma_start(out=aux_t, in_=aux_d.ap())
        cm_t = aux_t[:, 0, :]
        id_t = aux_t[:, 1, :]

        def group_bs(gg):
            return [min(gg * GB + j, n_batch - 1) for j in range(GB)]

        # W loads are issued two groups ahead of use, so on the SP queue they
        # sit in front of the piece stores of the group being computed —
        # otherwise those stores' copy-waits head-of-line-block ready loads
        # and starve the DMA engines
        w_t = {}

        def load_w(b):
            if b not in w_t:
                w_t[b] = wpool.tile([128, KC, N], BF16, tag="w", name="w_t")
                nc.sync.dma_start(out=w_t[b], in_=wts_r[:, b])

        def load_group_w(gg):
            if gg < n_groups:
                for b in sorted(set(group_bs(gg))):
                    load_w(b)

        load_group_w(0)
        load_group_w(1)
        load_w(group_bs(2)[0])
        load_w(group_bs(2)[1])

        ev = 0
        prev_dve_copies = []
        prev_act_copies = []
        for g in range(n_groups):
            bs = group_bs(g)
            emit = [g * GB + j < n_batch for j in range(GB)]
            dve_copies, act_copies = [], []

            # transposed features fT[hw, c] for mm2, one 128x128 transpose
            # per kc (slot j's fT lands at partition 32j automatically)
            ft_ps = ps_ft.tile([GP, KC, 128], BF16, name="ft_ps")
            for kc in range(KC):
                nc.tensor.transpose(
                    ft_ps[:, kc, :],
                    f_t[:, kc, bs[0] : bs[0] + GB, :],
                    id_t,
                )
            ft_sb = ftpool.tile([GP, KC, 128], BF16, name="ft_sb")
            nc.scalar.copy(ft_sb, ft_ps)

            # two halves of 2 n-chunks each: mm2 for a half only needs that
            # half's att chunks, so output stores stream mid-group instead of
            # bunching at the group end (shrinks the tail and group-boundary
            # DMA droughts)
            for h in range(2):
                # issue 2 of group g+2's weight loads at the top of each
                # half: the SP queue then alternates [2 loads, ~7 stores],
                # and each half's chunk-compute lull is covered by the loads
                # issued just ahead of the stores
                if g + 2 < n_groups:
                    bs2 = group_bs(g + 2)
                    load_w(bs2[2 * h])
                    load_w(bs2[2 * h + 1])
                att_h = attpool.tile([GP, 2, NW], BF16, tag="att")
                for q in range(2):
                    nb = 2 * h + q
                    sc_ps = ps_sc.tile([GP, NW], F32, name="sc_ps")
                    for j in range(GB):
                        for kc in range(KC):
                            mm = nc.tensor.matmul(
                                sc_ps[32 * j : 32 * j + 32, :],
                                f_t[:, kc, bs[j], :],
                                w_t[bs[j]][:, kc, nb * NW : (nb + 1) * NW],
                                start=(kc == 0),
                                stop=(kc == KC - 1),
                                tile_position=(0, 32 * j),
                            )
                            if g == n_groups - 1 and gate_inst is None:
                                gate_inst = mm
                            if g == 1 and h == 1 and gate2_inst is None:
                                gate2_inst = mm
                            if h == 0 and g not in first_mm1:
                                first_mm1[g] = mm
                    # softplus(x) = max(x,0) + ln(1 + exp(-|x|)): exp arg <= 0
                    # so no overflow, Ln input stays in [1,2]
                    t_abs = sppool.tile([GP, NW], F32, tag="tabs")
                    nc.scalar.activation(t_abs, sc_ps, AF.Abs)
                    t_exp = sppool.tile([GP, NW], F32, tag="texp")
                    nc.scalar.activation(t_exp, t_abs, AF.Exp, scale=-1.0)
                    t_ln = sppool.tile([GP, NW], F32, tag="tln")
                    nc.scalar.activation(t_ln, t_exp, AF.Ln, bias=1.0)
                    num_t = numpool.tile([GP, NW], BF16, tag="num")
                    nc.vector.scalar_tensor_tensor(
                        num_t, sc_ps, 0.0, t_ln, op0=ALU.max, op1=ALU.add
                    )
                    # fused reduce-over-hw + broadcast:
                    #   d[m] = sum_k C[k,m] num[k]
                    d_ps = ps_d.tile([GP, NW], F32, name="d_ps")
                    nc.tensor.matmul(d_ps, cm_t, num_t, start=True, stop=True)
                    r_t = rpool.tile([GP, NW], F32, tag="r")
                    nc.vector.reciprocal(r_t, d_ps)
                    nc.vector.tensor_tensor(
                        att_h[:, q, :], num_t, r_t, op=ALU.mult
                    )

                for j in range(GB):
                    if not emit[j]:
                        continue
                    for kc in range(KC):
                        o_ps = ps_o.tile([128, 2, NW], F32)
                        for q in range(2):
                            nc.tensor.matmul(
                                o_ps[:, q, :],
                                ft_sb[32 * j : 32 * j + HW, kc, :],
                                att_h[32 * j : 32 * j + HW, q, :],
                                start=True,
                                stop=True,
                                tile_position=(32 * j, 0),
                            )
                        # stash some early-group pieces: their DMAs are
                        # deferred to the end of the program to keep the DMA
                        # engines fed while the last group's compute trickles
                        # out its stores
                        late = (
                            g < n_groups - 1 and kc == 0 and (j == 0 or (j == 1 and h == 0))
                        )
                        # a small early wave from group 0 bridges the
                        # load-burst -> steady-state DMA lull around t~50us;
                        # one mid-wave piece per group fills the recurring
                        # group-boundary lull two groups later
                        early = g == 0 and kc == 1 and (j == 0 or (j == 1 and h == 0))
                        mid = (
                            g + 3 < n_groups and kc == 1 and j == 1 and h == 1
                        )
                        stash_this = late or early or mid
                        dst_ap = out_r[:, bs[j], kc, 2 * h * NW : 2 * h * NW + 2 * NW]
                        if stash_this:
                            po = stashpool.tile(
                                [128, 2, NW], BF16, tag="stash", name="stash"
                            )
                            # stash wave order key: early=~g1, mid=g+2, late=g7
                            wave = 1 if early else (g + 2 if mid else n_groups - 1)
                            stash.append((po, dst_ap, wave))
                        else:
                            po = opool.tile([128, 2, NW], BF16, tag="po", name="po")
                        if ev % 2 == 0:
                            act_copies.append(nc.scalar.copy(po, o_ps))
                        else:
                            dve_copies.append(nc.vector.tensor_copy(po, o_ps))
                        ev += 1
                        if not stash_this:
                            # stream each [128,1024] piece out independently
                            nc.sync.dma_start(out=dst_ap, in_=po)

            prev_dve_copies, prev_act_copies = dve_copies, act_copies

        # deferred stash stores on the idle Pool/SWDGE queue.  The early wave
        # is gated on group 1's second-half matmuls (fills the load-burst ->
        # steady-state lull); the late wave on the last group's first matmul
        # (fills the tail while the final group's stores trickle through its
        # compute chain)
        for po, dst, wave in sorted(stash, key=lambda s: s[2]):
            d = nc.gpsimd.dma_start(out=dst, in_=po)
            gate = gate2_inst if wave == 1 else first_mm1[wave]
            tile.add_dep_helper(
                d.ins, gate.ins, sync=True,
                reason="defer stash stores into DMA lulls",
            )

    nc.compile()
    return nc


_NC_CACHE = {}


def _get_nc(n_batch=B):
    if n_batch not in _NC_CACHE:
        _NC_CACHE[n_batch] = build_nc(n_batch)
    return _NC_CACHE[n_batch]


def prep_features(features):
    """[nb, C, H, W] f32 -> padded bf16 [128, KC, nb, 32]."""
    features = np.asarray(features, dtype=np.float32)
    nb = features.shape[0]
    f4 = features.reshape(nb, KC, 128, HW)
    fpad = np.zeros((nb, KC, 128, 32), np.float32)
    fpad[..., :HW] = f4
    return np.ascontiguousarray(fpad.transpose(2, 1, 0, 3)).astype(BF_NP)


def run(features, weights, trace=False, **kwargs):
    """Shard over 8 cores, run, gather. Returns (out, BassKernelResults)."""
    fpad = prep_features(features)
    weights = np.asarray(weights, dtype=np.float32).astype(BF_NP)
    aux = aux_inputs()
    nc = _get_nc()
    in_maps = []
    for i in range(N_CORES):
        sl = slice(i * B, (i + 1) * B)
        in_maps.append({"fpad": fpad[:, :, sl], "weights": weights[sl], **aux})
    res = run_bass_kernel_spmd(
        nc, in_maps, core_ids=list(range(N_CORES)), trace=trace, **kwargs
    )
    out = np.concatenate(
        [np.asarray(r["out"]).astype(np.float32) for r in res.results], axis=0
    )
    return out, res


def kernel(features, weights):
    out, _ = run(features, weights)
    return out


# revision 68
# speedup vs baseline: 2.8294x; 1.1595x over previous
"""Attentional pooling layer on Trainium2 (Bass/Tile), 8-core batch-parallel.

Reference computation per batch b:
    scores[hw, n] = sum_c f[c, hw] * w[c, n]          (mm1)
    num           = softplus(scores)                  (ACT: Abs/Exp/Ln)
    denom[n]      = sum_hw num[hw, n]                 (PE reduce)
    att[hw, n]    = num / denom[n]                    (DVE recip + mult)
    out[c, n]     = sum_hw f[c, hw] * att[hw, n]      (mm2)

All dense traffic is bf16 (weights in, outputs out) — the rel-err budget
(2e-2) dwarfs bf16 rounding (~4e-3 end to end), and HBM time is the
bottleneck, so halving bytes halves the roofline.  The +1e-4 numerator /
+16e-4 denominator constants of the reference are dropped entirely
(measured end-to-end impact 5.5e-5).

Partition layout: 4 batches per group at 32-partition offsets 0/32/64/96
(explicit tile_position, bypassing the conservative 0/32/64 helper assert;
base 96 verified on hardware).  32 batches per core = 8 exact groups.

The denominator reduce-over-hw AND its broadcast back to all 128
partitions are fused into ONE matmul with a constant 0/1 matrix
C[k, m] = 1 iff k, m in the same 32-block and k%32 < 16.  A single act
table (natural_log_exp_and_others: Abs+Exp+Ln+Copy) serves every ACT op,
so exactly one table load is emitted — forced by blanking the other table
entries (indices preserved, so act_func_set_id still matches walrus's
act_info.json).

Output drain PSUM->SBUF casts f32->bf16 on DVE/ACT (GPSIMD cannot touch
PSUM), in [128,1024] units (2 PSUM banks) to amortize access latency.
"""

import numpy as np
import ml_dtypes
from contextlib import ExitStack

import concourse.bass as bass
import concourse.bacc as bacc
import concourse.tile as tile
from concourse import mybir
from concourse.bass_utils import run_bass_kernel_spmd

F32 = mybir.dt.float32
BF16 = mybir.dt.bfloat16
F8E3 = mybir.dt.float8e3
AF = mybir.ActivationFunctionType
ALU = mybir.AluOpType
BF_NP = ml_dtypes.bfloat16

# Weights travel as fp8 e3m4 (1 byte), halving the weight-load traffic again.
# The PE multiplies fp8 operands exactly, so only the quantization of w
# matters: end-to-end rel err 1.50e-2 vs the 2e-2 gate (measured, seed-fixed,
# deterministic).  w is pre-scaled by WS=2.5 into e3m4's [~0.008, 15.5]
# band; features carry 1/WS so scores come out exact, and the denominator
# matrix carries 1/WS so mm2's (f/WS) @ (WS*att) cancels exactly.
WS = 2.5

# Force every ACT op onto one table so only a single table load is emitted.
# Keys/positions are preserved (ids index walrus's act_info.json); only the
# *selection* changes: all other tables are presented as empty so the greedy
# insert pass picks natural_log_exp_and_others (Abs+Exp+Ln+Copy) for all.
_KEEP_TABLE = "natural_log_exp_and_others"
_orig_get_tables = bacc.get_activation_tables


def _single_table(arch):
    tbls = _orig_get_tables(arch)
    return {k: (v if k == _KEEP_TABLE else set()) for k, v in tbls.items()}


bacc.get_activation_tables = _single_table

N_CORES = 8
B_FULL, C, H, W, N = 256, 256, 4, 4, 2048
HW = H * W                  # 16
B = B_FULL // N_CORES       # 32 batches per core
KC = C // 128               # 2 contraction chunks of 128
GB = 4                      # batches per partition group (offsets 0/32/64/96)
GP = 32 * GB                # 128 partitions per group
NCH = 4                     # n chunks per group
NW = N // NCH               # 512 (one PSUM bank)


def aux_inputs():
    # aux[:, 0, :] = cmat: cmat[k, m] = 1 iff k and m share a 32-block and
    # k%32 is a real hw row; (C^T @ num) both reduces over hw and broadcasts
    # back to 128 partitions.  aux[:, 1, :] = 128x128 identity (transposes).
    cmat = np.zeros((GP, GP), np.float32)
    for k in range(GP):
        for m in range(GP):
            if k // 32 == m // 32 and k % 32 < HW:
                cmat[k, m] = 1.0 / WS
    aux = np.stack([cmat, np.eye(128, dtype=np.float32)], axis=1)
    return {"aux": aux.astype(BF_NP)}


def build_nc(n_batch=B, debug=False):
    nc = bacc.Bacc(None, target_bir_lowering=False, debug=debug)
    feat = nc.dram_tensor("fpad", [128, KC, n_batch, 32], BF16, kind="ExternalInput")
    wts = nc.dram_tensor("weights", [n_batch, C, N], F8E3, kind="ExternalInput")
    out = nc.dram_tensor("out", [n_batch, C, N], BF16, kind="ExternalOutput")
    aux_d = nc.dram_tensor("aux", [128, 2, 128], BF16, kind="ExternalInput")

    # [ci, b, kc, n] views of the DRAM tensors
    wts_r = wts.ap().rearrange("b (kc ci) n -> ci b kc n", kc=KC)
    out_r = out.ap().rearrange("b (kc ci) n -> ci b kc n", kc=KC)

    n_groups = (n_batch + GB - 1) // GB

    with tile.TileContext(nc) as tc, ExitStack() as ctx:
        singles = ctx.enter_context(tc.tile_pool(name="singles", bufs=1))
        wpool = ctx.enter_context(tc.tile_pool(name="w", bufs=18))
        opool = ctx.enter_context(tc.tile_pool(name="o", bufs=6))
        stashpool = ctx.enter_context(tc.tile_pool(name="stash", bufs=39))
        stash = []
        gate_inst = None
        gate2_inst = None
        first_mm1 = {}
        sppool = ctx.enter_context(tc.tile_pool(name="sp", bufs=2))
        numpool = ctx.enter_context(tc.tile_pool(name="num", bufs=2))
        rpool = ctx.enter_context(tc.tile_pool(name="r", bufs=2))
        attpool = ctx.enter_context(tc.tile_pool(name="att", bufs=2))
        ftpool = ctx.enter_context(tc.tile_pool(name="ft", bufs=2))
        ps_sc = ctx.enter_context(tc.tile_pool(name="ps_sc", bufs=2, space="PSUM"))
        ps_d = ctx.enter_context(tc.tile_pool(name="ps_d", bufs=1, space="PSUM"))
        ps_ft = ctx.enter_context(tc.tile_pool(name="ps_ft", bufs=1, space="PSUM"))
        ps_o = ctx.enter_context(tc.tile_pool(name="ps_o", bufs=2, space="PSUM"))

        # features, pre-transposed + hw-padded to 32 with zeros on the host
        f_t = singles.tile([128, KC, n_batch, 32], BF16)
        nc.sync.dma_start(out=f_t, in_=feat.ap())

        aux_t = singles.tile([128, 2, 128], BF16)
        nc.sync.dma_start(out=aux_t, in_=aux_d.ap())
        cm_t = aux_t[:, 0, :]
        id_t = aux_t[:, 1, :]

        def group_bs(gg):
            return [min(gg * GB + j, n_batch - 1) for j in range(GB)]

        # W loads are issued two groups ahead of use, so on the SP queue they
        # sit in front of the piece stores of the group being computed —
        # otherwise those stores' copy-waits head-of-line-block ready loads
        # and starve the DMA engines
        w_t = {}

        def load_w(b):
            if b not in w_t:
                w_t[b] = wpool.tile([128, KC, N], F8E3, tag="w", name="w_t")
                nc.sync.dma_start(out=w_t[b], in_=wts_r[:, b])

        def load_group_w(gg):
            if gg < n_groups:
                for b in sorted(set(group_bs(gg))):
                    load_w(b)

        load_group_w(0)
        load_group_w(1)
        load_group_w(2)
        load_group_w(3)

        ev = 0
        prev_dve_copies = []
        prev_act_copies = []
        for g in range(n_groups):
            bs = group_bs(g)
            emit = [g * GB + j < n_batch for j in range(GB)]
            dve_copies, act_copies = [], []

            # transposed features fT[hw, c] for mm2, one 128x128 transpose
            # per kc (slot j's fT lands at partition 32j automatically)
            ft_ps = ps_ft.tile([GP, KC, 128], BF16, name="ft_ps")
            for kc in range(KC):
                nc.tensor.transpose(
                    ft_ps[:, kc, :],
                    f_t[:, kc, bs[0] : bs[0] + GB, :],
                    id_t,
                )
            ft_sb = ftpool.tile([GP, KC, 128], BF16, name="ft_sb")
            nc.scalar.copy(ft_sb, ft_ps)

            # two halves of 2 n-chunks each: mm2 for a half only needs that
            # half's att chunks, so output stores stream mid-group instead of
            # bunching at the group end (shrinks the tail and group-boundary
            # DMA droughts)
            for h in range(2):
                # issue 2 of group g+2's weight loads at the top of each
                # half: the SP queue then alternates [2 loads, ~7 stores],
                # and each half's chunk-compute lull is covered by the loads
                # issued just ahead of the stores
                if g + 4 < n_groups:
                    bs2 = group_bs(g + 4)
                    load_w(bs2[2 * h])
                    load_w(bs2[2 * h + 1])
                att_h = attpool.tile([GP, 2, NW], BF16, tag="att")
                for q in range(2):
                    nb = 2 * h + q
                    sc_ps = ps_sc.tile([GP, NW], F32, name="sc_ps")
                    for j in range(GB):
                        for kc in range(KC):
                            mm = nc.tensor.matmul(
                                sc_ps[32 * j : 32 * j + 32, :],
                                f_t[:, kc, bs[j], :],
                                w_t[bs[j]][:, kc, nb * NW : (nb + 1) * NW],
                                start=(kc == 0),
                                stop=(kc == KC - 1),
                                tile_position=(0, 32 * j),
                            )
                            if g == n_groups - 1 and gate_inst is None:
                                gate_inst = mm
                            if g == 1 and h == 1 and gate2_inst is None:
                                gate2_inst = mm
                            if h == 0 and g not in first_mm1:
                                first_mm1[g] = mm
                    # softplus(x) = max(x,0) + ln(1 + exp(-|x|)): exp arg <= 0
                    # so no overflow, Ln input stays in [1,2]
                    t_abs = sppool.tile([GP, NW], F32, tag="tabs")
                    nc.scalar.activation(t_abs, sc_ps, AF.Abs)
                    t_exp = sppool.tile([GP, NW], F32, tag="texp")
                    nc.scalar.activation(t_exp, t_abs, AF.Exp, scale=-1.0)
                    t_ln = sppool.tile([GP, NW], F32, tag="tln")
                    nc.scalar.activation(t_ln, t_exp, AF.Ln, bias=1.0)
                    num_t = numpool.tile([GP, NW], BF16, tag="num")
                    nc.vector.scalar_tensor_tensor(
                        num_t, sc_ps, 0.0, t_ln, op0=ALU.max, op1=ALU.add
                    )
                    # fused reduce-over-hw + broadcast:
                    #   d[m] = sum_k C[k,m] num[k]
                    d_ps = ps_d.tile([GP, NW], F32, name="d_ps")
                    nc.tensor.matmul(d_ps, cm_t, num_t, start=True, stop=True)
                    r_t = rpool.tile([GP, NW], F32, tag="r")
                    nc.vector.reciprocal(r_t, d_ps)
                    # att = num * (1/d): all-SBUF, so it can run on the
                    # otherwise-idle GPSIMD engine instead of DVE
                    nc.gpsimd.tensor_tensor(
                        att_h[:, q, :], num_t, r_t, op=ALU.mult
                    )

                for j in range(GB):
                    if not emit[j]:
                        continue
                    for kc in range(KC):
                        o_ps = ps_o.tile([128, 2, NW], F32)
                        for q in range(2):
                            nc.tensor.matmul(
                                o_ps[:, q, :],
                                ft_sb[32 * j : 32 * j + HW, kc, :],
                                att_h[32 * j : 32 * j + HW, q, :],
                                start=True,
                                stop=True,
                                tile_position=(32 * j, 0),
                            )
                        # stash some early-group pieces: their DMAs are
                        # deferred to the end of the program to keep the DMA
                        # engines fed while the last group's compute trickles
                        # out its stores
                        late = (
                            g < n_groups - 1 and kc == 0 and (j == 0 or (j == 1 and h == 0))
                        )
                        # a small early wave from group 0 bridges the
                        # load-burst -> steady-state DMA lull around t~50us;
                        # one mid-wave piece per group fills the recurring
                        # group-boundary lull two groups later
                        early = g == 0 and kc == 1 and (j == 0 or (j == 1 and h == 0))
                        mid = g + 3 < n_groups and kc == 1 and (
                            (j in (1, 2) and h == 1) or (j == 2 and h == 0)
                        )
                        stash_this = late or early or mid
                        dst_ap = out_r[:, bs[j], kc, 2 * h * NW : 2 * h * NW + 2 * NW]
                        if stash_this:
                            po = stashpool.tile(
                                [128, 2, NW], BF16, tag="stash", name="stash"
                            )
                            # stash wave order key: early=~g1, mid=g+2, late=g7
                            wave = 1 if early else (g + 2 if mid else n_groups - 1)
                            stash.append((po, dst_ap, wave))
                        else:
                            po = opool.tile([128, 2, NW], BF16, tag="po", name="po")
                        # strictly alternate ACT/DVE: same-engine runs
                        # serialize the 2-buffer o_ps PSUM rotation
                        if ev % 2 == 0:
                            act_copies.append(nc.scalar.copy(po, o_ps))
                        else:
                            dve_copies.append(nc.vector.tensor_copy(po, o_ps))
                        ev += 1
                        if not stash_this:
                            # stream each [128,1024] piece out independently
                            nc.sync.dma_start(out=dst_ap, in_=po)

            prev_dve_copies, prev_act_copies = dve_copies, act_copies

        # deferred stash stores on the idle Pool/SWDGE queue.  The early wave
        # is gated on group 1's second-half matmuls (fills the load-burst ->
        # steady-state lull); the late wave on the last group's first matmul
        # (fills the tail while the final group's stores trickle through its
        # compute chain)
        for po, dst, wave in sorted(stash, key=lambda s: s[2]):
            d = nc.gpsimd.dma_start(out=dst, in_=po)
            gate = gate2_inst if wave == 1 else first_mm1[wave]
            tile.add_dep_helper(
                d.ins, gate.ins, sync=True,
                reason="defer stash stores into DMA lulls",
            )

    nc.compile()
    return nc


_NC_CACHE = {}


def _get_nc(n_batch=B):
    if n_batch not in _NC_CACHE:
        _NC_CACHE[n_batch] = build_nc(n_batch)
    return _NC_CACHE[n_batch]


def prep_features(features):
    """[nb, C, H, W] f32 -> padded bf16 [128, KC, nb, 32], pre-scaled 1/WS."""
    features = np.asarray(features, dtype=np.float32) * (1.0 / WS)
    nb = features.shape[0]
    f4 = features.reshape(nb, KC, 128, HW)
    fpad = np.zeros((nb, KC, 128, 32), np.float32)
    fpad[..., :HW] = f4
    return np.ascontiguousarray(fpad.transpose(2, 1, 0, 3)).astype(BF_NP)


def run(features, weights, trace=False, **kwargs):
    """Shard over 8 cores, run, gather. Returns (out, BassKernelResults)."""
    fpad = prep_features(features)
    weights = (np.asarray(weights, dtype=np.float32) * WS).astype(
        ml_dtypes.float8_e3m4
    )
    aux = aux_inputs()
    nc = _get_nc()
    in_maps = []
    for i in range(N_CORES):
        sl = slice(i * B, (i + 1) * B)
        in_maps.append({"fpad": fpad[:, :, sl], "weights": weights[sl], **aux})
    res = run_bass_kernel_spmd(
        nc, in_maps, core_ids=list(range(N_CORES)), trace=trace, **kwargs
    )
    out = np.concatenate(
        [np.asarray(r["out"]).astype(np.float32) for r in res.results], axis=0
    )
    return out, res


def kernel(features, weights):
    out, _ = run(features, weights)
    return out


# revision 70
# speedup vs baseline: 2.8327x; 1.0012x over previous
"""Attentional pooling layer on Trainium2 (Bass/Tile), 8-core batch-parallel.

Reference computation per batch b:
    scores[hw, n] = sum_c f[c, hw] * w[c, n]          (mm1)
    num           = softplus(scores)                  (ACT: Abs/Exp/Ln)
    denom[n]      = sum_hw num[hw, n]                 (PE reduce)
    att[hw, n]    = num / denom[n]                    (DVE recip + mult)
    out[c, n]     = sum_hw f[c, hw] * att[hw, n]      (mm2)

HBM time is the bottleneck, so bytes are minimized against the rel-err
budget (2e-2): weights travel as fp8 e3m4 (see WS below), activations and
outputs as bf16.  Measured end-to-end rel err: 1.49e-2 (seed-fixed,
deterministic).  The +1e-4 numerator / +16e-4 denominator constants of the
reference are dropped entirely (measured end-to-end impact 5.5e-5).

Partition layout: 4 batches per group at 32-partition offsets 0/32/64/96
(explicit tile_position, bypassing the conservative 0/32/64 helper assert;
base 96 verified on hardware).  32 batches per core = 8 exact groups.

The denominator reduce-over-hw AND its broadcast back to all 128
partitions are fused into ONE matmul with a constant 0/1 matrix
C[k, m] = 1 iff k, m in the same 32-block and k%32 < 16.  A single act
table (natural_log_exp_and_others: Abs+Exp+Ln+Copy) serves every ACT op,
so exactly one table load is emitted — forced by blanking the other table
entries (indices preserved, so act_func_set_id still matches walrus's
act_info.json).

Output drain PSUM->SBUF casts f32->bf16 on DVE/ACT (GPSIMD cannot touch
PSUM), in [128,1024] units (2 PSUM banks) to amortize access latency.
"""

import numpy as np
import ml_dtypes
from contextlib import ExitStack

import concourse.bass as bass
import concourse.bacc as bacc
import concourse.tile as tile
from concourse import mybir
from concourse.bass_utils import run_bass_kernel_spmd

F32 = mybir.dt.float32
BF16 = mybir.dt.bfloat16
F8E3 = mybir.dt.float8e3
AF = mybir.ActivationFunctionType
ALU = mybir.AluOpType
BF_NP = ml_dtypes.bfloat16

# Weights travel as fp8 e3m4 (1 byte), halving the weight-load traffic again.
# The PE multiplies fp8 operands exactly, so only the quantization of w
# matters: end-to-end rel err 1.50e-2 vs the 2e-2 gate (measured, seed-fixed,
# deterministic).  w is pre-scaled by WS=2.5 into e3m4's [~0.008, 15.5]
# band; features carry 1/WS so scores come out exact, and the denominator
# matrix carries 1/WS so mm2's (f/WS) @ (WS*att) cancels exactly.
WS = 2.5

# Force every ACT op onto one table so only a single table load is emitted.
# Keys/positions are preserved (ids index walrus's act_info.json); only the
# *selection* changes: all other tables are presented as empty so the greedy
# insert pass picks natural_log_exp_and_others (Abs+Exp+Ln+Copy) for all.
_KEEP_TABLE = "natural_log_exp_and_others"
_orig_get_tables = bacc.get_activation_tables


def _single_table(arch):
    tbls = _orig_get_tables(arch)
    return {k: (v if k == _KEEP_TABLE else set()) for k, v in tbls.items()}


bacc.get_activation_tables = _single_table

N_CORES = 8
B_FULL, C, H, W, N = 256, 256, 4, 4, 2048
HW = H * W                  # 16
B = B_FULL // N_CORES       # 32 batches per core
KC = C // 128               # 2 contraction chunks of 128
GB = 4                      # batches per partition group (offsets 0/32/64/96)
GP = 32 * GB                # 128 partitions per group
NCH = 4                     # n chunks per group
NW = N // NCH               # 512 (one PSUM bank)


def aux_inputs():
    # aux[:, 0, :] = cmat: cmat[k, m] = 1 iff k and m share a 32-block and
    # k%32 is a real hw row; (C^T @ num) both reduces over hw and broadcasts
    # back to 128 partitions.  aux[:, 1, :] = 128x128 identity (transposes).
    cmat = np.zeros((GP, GP), np.float32)
    for k in range(GP):
        for m in range(GP):
            if k // 32 == m // 32 and k % 32 < HW:
                cmat[k, m] = 1.0 / WS
    aux = np.stack([cmat, np.eye(128, dtype=np.float32)], axis=1)
    return {"aux": aux.astype(BF_NP)}


def build_nc(n_batch=B, debug=False):
    nc = bacc.Bacc(None, target_bir_lowering=False, debug=debug)
    feat = nc.dram_tensor("fpad", [128, KC, n_batch, 32], BF16, kind="ExternalInput")
    wts = nc.dram_tensor("weights", [n_batch, C, N], F8E3, kind="ExternalInput")
    out = nc.dram_tensor("out", [n_batch, C, N], BF16, kind="ExternalOutput")
    aux_d = nc.dram_tensor("aux", [128, 2, 128], BF16, kind="ExternalInput")

    # [ci, b, kc, n] views of the DRAM tensors
    wts_r = wts.ap().rearrange("b (kc ci) n -> ci b kc n", kc=KC)
    out_r = out.ap().rearrange("b (kc ci) n -> ci b kc n", kc=KC)

    n_groups = (n_batch + GB - 1) // GB

    with tile.TileContext(nc) as tc, ExitStack() as ctx:
        singles = ctx.enter_context(tc.tile_pool(name="singles", bufs=1))
        wpool = ctx.enter_context(tc.tile_pool(name="w", bufs=18))
        opool = ctx.enter_context(tc.tile_pool(name="o", bufs=6))
        stashpool = ctx.enter_context(tc.tile_pool(name="stash", bufs=44))
        stash = []
        gate_inst = None
        gate2_inst = None
        first_mm1 = {}
        sppool = ctx.enter_context(tc.tile_pool(name="sp", bufs=2))
        numpool = ctx.enter_context(tc.tile_pool(name="num", bufs=2))
        rpool = ctx.enter_context(tc.tile_pool(name="r", bufs=2))
        attpool = ctx.enter_context(tc.tile_pool(name="att", bufs=2))
        ftpool = ctx.enter_context(tc.tile_pool(name="ft", bufs=2))
        ps_sc = ctx.enter_context(tc.tile_pool(name="ps_sc", bufs=2, space="PSUM"))
        ps_d = ctx.enter_context(tc.tile_pool(name="ps_d", bufs=1, space="PSUM"))
        ps_ft = ctx.enter_context(tc.tile_pool(name="ps_ft", bufs=1, space="PSUM"))
        ps_o = ctx.enter_context(tc.tile_pool(name="ps_o", bufs=2, space="PSUM"))

        # features, pre-transposed + hw-padded to 32 with zeros on the host
        f_t = singles.tile([128, KC, n_batch, 32], BF16)
        nc.sync.dma_start(out=f_t, in_=feat.ap())

        aux_t = singles.tile([128, 2, 128], BF16)
        nc.sync.dma_start(out=aux_t, in_=aux_d.ap())
        cm_t = aux_t[:, 0, :]
        id_t = aux_t[:, 1, :]

        def group_bs(gg):
            return [min(gg * GB + j, n_batch - 1) for j in range(GB)]

        # W loads are issued two groups ahead of use, so on the SP queue they
        # sit in front of the piece stores of the group being computed —
        # otherwise those stores' copy-waits head-of-line-block ready loads
        # and starve the DMA engines
        w_t = {}

        def load_w(b):
            if b not in w_t:
                w_t[b] = wpool.tile([128, KC, N], F8E3, tag="w", name="w_t")
                nc.sync.dma_start(out=w_t[b], in_=wts_r[:, b])

        def load_group_w(gg):
            if gg < n_groups:
                for b in sorted(set(group_bs(gg))):
                    load_w(b)

        load_group_w(0)
        load_group_w(1)
        load_group_w(2)
        load_group_w(3)

        ev = 0
        prev_dve_copies = []
        prev_act_copies = []
        for g in range(n_groups):
            bs = group_bs(g)
            emit = [g * GB + j < n_batch for j in range(GB)]
            dve_copies, act_copies = [], []

            # transposed features fT[hw, c] for mm2, one 128x128 transpose
            # per kc (slot j's fT lands at partition 32j automatically)
            ft_ps = ps_ft.tile([GP, KC, 128], BF16, name="ft_ps")
            for kc in range(KC):
                nc.tensor.transpose(
                    ft_ps[:, kc, :],
                    f_t[:, kc, bs[0] : bs[0] + GB, :],
                    id_t,
                )
            ft_sb = ftpool.tile([GP, KC, 128], BF16, name="ft_sb")
            nc.scalar.copy(ft_sb, ft_ps)

            # two halves of 2 n-chunks each: mm2 for a half only needs that
            # half's att chunks, so output stores stream mid-group instead of
            # bunching at the group end (shrinks the tail and group-boundary
            # DMA droughts)
            for h in range(2):
                # issue 2 of group g+2's weight loads at the top of each
                # half: the SP queue then alternates [2 loads, ~7 stores],
                # and each half's chunk-compute lull is covered by the loads
                # issued just ahead of the stores
                if g + 4 < n_groups:
                    bs2 = group_bs(g + 4)
                    load_w(bs2[2 * h])
                    load_w(bs2[2 * h + 1])
                att_h = attpool.tile([GP, 2, NW], BF16, tag="att")
                for q in range(2):
                    nb = 2 * h + q
                    sc_ps = ps_sc.tile([GP, NW], F32, name="sc_ps")
                    for j in range(GB):
                        for kc in range(KC):
                            mm = nc.tensor.matmul(
                                sc_ps[32 * j : 32 * j + 32, :],
                                f_t[:, kc, bs[j], :],
                                w_t[bs[j]][:, kc, nb * NW : (nb + 1) * NW],
                                start=(kc == 0),
                                stop=(kc == KC - 1),
                                tile_position=(0, 32 * j),
                            )
                            if g == n_groups - 1 and gate_inst is None:
                                gate_inst = mm
                            if g == 1 and h == 1 and gate2_inst is None:
                                gate2_inst = mm
                            if h == 0 and g not in first_mm1:
                                first_mm1[g] = mm
                    # softplus(x) = max(x,0) + ln(1 + exp(-|x|)): exp arg <= 0
                    # so no overflow, Ln input stays in [1,2]
                    t_abs = sppool.tile([GP, NW], F32, tag="tabs")
                    nc.scalar.activation(t_abs, sc_ps, AF.Abs)
                    t_exp = sppool.tile([GP, NW], F32, tag="texp")
                    nc.scalar.activation(t_exp, t_abs, AF.Exp, scale=-1.0)
                    t_ln = sppool.tile([GP, NW], F32, tag="tln")
                    nc.scalar.activation(t_ln, t_exp, AF.Ln, bias=1.0)
                    num_t = numpool.tile([GP, NW], BF16, tag="num")
                    nc.vector.scalar_tensor_tensor(
                        num_t, sc_ps, 0.0, t_ln, op0=ALU.max, op1=ALU.add
                    )
                    # fused reduce-over-hw + broadcast:
                    #   d[m] = sum_k C[k,m] num[k]
                    d_ps = ps_d.tile([GP, NW], F32, name="d_ps")
                    nc.tensor.matmul(d_ps, cm_t, num_t, start=True, stop=True)
                    r_t = rpool.tile([GP, NW], F32, tag="r")
                    nc.vector.reciprocal(r_t, d_ps)
                    # att = num * (1/d): all-SBUF, so it can run on the
                    # otherwise-idle GPSIMD engine instead of DVE
                    nc.gpsimd.tensor_tensor(
                        att_h[:, q, :], num_t, r_t, op=ALU.mult
                    )

                for j in range(GB):
                    if not emit[j]:
                        continue
                    for kc in range(KC):
                        o_ps = ps_o.tile([128, 2, NW], F32)
                        for q in range(2):
                            nc.tensor.matmul(
                                o_ps[:, q, :],
                                ft_sb[32 * j : 32 * j + HW, kc, :],
                                att_h[32 * j : 32 * j + HW, q, :],
                                start=True,
                                stop=True,
                                tile_position=(32 * j, 0),
                            )
                        # stash some early-group pieces: their DMAs are
                        # deferred to the end of the program to keep the DMA
                        # engines fed while the last group's compute trickles
                        # out its stores
                        late = (
                            g < n_groups - 1 and kc == 0 and (j == 0 or (j == 1 and h == 0))
                        )
                        # a small early wave from group 0 bridges the
                        # load-burst -> steady-state DMA lull around t~50us;
                        # one mid-wave piece per group fills the recurring
                        # group-boundary lull two groups later
                        early = g == 0 and kc == 1 and (j == 0 or (j == 1 and h == 0))
                        mid = g + 3 < n_groups and kc == 1 and (
                            (j in (1, 2) and h == 1) or (j in (1, 2) and h == 0)
                        )
                        stash_this = late or early or mid
                        dst_ap = out_r[:, bs[j], kc, 2 * h * NW : 2 * h * NW + 2 * NW]
                        if stash_this:
                            po = stashpool.tile(
                                [128, 2, NW], BF16, tag="stash", name="stash"
                            )
                            # stash wave order key: early=~g1, mid=g+2, late=g7
                            wave = 1 if early else (g + 2 if mid else n_groups - 1)
                            stash.append((po, dst_ap, wave))
                        else:
                            po = opool.tile([128, 2, NW], BF16, tag="po", name="po")
                        # strictly alternate ACT/DVE: same-engine runs
                        # serialize the 2-buffer o_ps PSUM rotation
                        if ev % 2 == 0:
                            act_copies.append(nc.scalar.copy(po, o_ps))
                        else:
                            dve_copies.append(nc.vector.tensor_copy(po, o_ps))
                        ev += 1
                        if not stash_this:
                            # stream each [128,1024] piece out independently
                            nc.sync.dma_start(out=dst_ap, in_=po)

            prev_dve_copies, prev_act_copies = dve_copies, act_copies

        # deferred stash stores on the idle Pool/SWDGE queue.  The early wave
        # is gated on group 1's second-half matmuls (fills the load-burst ->
        # steady-state lull); the late wave on the last group's first matmul
        # (fills the tail while the final group's stores trickle through its
        # compute chain)
        for po, dst, wave in sorted(stash, key=lambda s: s[2]):
            d = nc.gpsimd.dma_start(out=dst, in_=po)
            gate = gate2_inst if wave == 1 else first_mm1[wave]
            tile.add_dep_helper(
                d.ins, gate.ins, sync=True,
                reason="defer stash stores into DMA lulls",
            )

    nc.compile()
    return nc


_NC_CACHE = {}


def _get_nc(n_batch=B):
    if n_batch not in _NC_CACHE:
        _NC_CACHE[n_batch] = build_nc(n_batch)
    return _NC_CACHE[n_batch]


def prep_features(features):
    """[nb, C, H, W] f32 -> padded bf16 [128, KC, nb, 32], pre-scaled 1/WS."""
    features = np.asarray(features, dtype=np.float32) * (1.0 / WS)
    nb = features.shape[0]
    f4 = features.reshape(nb, KC, 128, HW)
    fpad = np.zeros((nb, KC, 128, 32), np.float32)
    fpad[..., :HW] = f4
    return np.ascontiguousarray(fpad.transpose(2, 1, 0, 3)).astype(BF_NP)


def run(features, weights, trace=False, **kwargs):
    """Shard over 8 cores, run, gather. Returns (out, BassKernelResults)."""
    fpad = prep_features(features)
    weights = (np.asarray(weights, dtype=np.float32) * WS).astype(
        ml_dtypes.float8_e3m4
    )
    aux = aux_inputs()
    nc = _get_nc()
    in_maps = []
    for i in range(N_CORES):
        sl = slice(i * B, (i + 1) * B)
        in_maps.append({"fpad": fpad[:, :, sl], "weights": weights[sl], **aux})
    res = run_bass_kernel_spmd(
        nc, in_maps, core_ids=list(range(N_CORES)), trace=trace, **kwargs
    )
    out = np.concatenate(
        [np.asarray(r["out"]).astype(np.float32) for r in res.results], axis=0
    )
    return out, res


def kernel(features, weights):
    out, _ = run(features, weights)
    return out
